# revision 19
# baseline (speedup 1.0000x reference)
"""Trainium2 Bass kernel for nn_Attention_13778255085887.

Dense multi-head attention block (EfficientViT-style):
  qkv 1x1 conv -> per-head softmax(q^T k * scale) -> v @ attn^T
  + depthwise conv(k=3) positional encoding on v -> proj 1x1 conv.

Shapes: B=8, dim=256, L=1024, heads=8, key_dim=16, head_dim=32.

Strategy: data-parallel over B across the 8 NeuronCores (zero collectives).
The serial bottleneck is the 8.4M-element softmax exp per core.  It is split
across TWO engines running concurrently:

  - ScalarE runs exact ACTIVATE-Exp on ~60% of the (128, 1024) score tiles.
  - VectorE runs a Schraudolph-style exp on the rest: one fused
    tensor_scalar  int16(round(x * 0.25*log2e*128 + (16256 - 5.5)))  whose
    int16 bit pattern IS the bf16 encoding of 2^(0.25*x*log2e) ~ exp(x/4)
    (max ~3% elementwise error; softmax weights are near-uniform at this
    scale so the error averages out ~30x below the 2e-2 gate -- verified in
    numpy simulation end-to-end at ~7e-3 vs 3.6e-3 all-exact).

  - q/k are projected into a packed layout (4 heads per 128-partition tile,
    head g at partitions 32g..32g+16, zero-padded to 32) so the tiny K=16
    score matmuls run 4-at-a-time via tile_position row groups.
  - S^T = k^T q is built per (head-pair, j-chunk) directly with j on
    partitions; softmax-without-max (logits bounded ~[-1.5, 1.5]) makes
    exp a single instruction per (128, 1024) PSUM block.
  - v^T (AV stationary operand) is computed directly as x^T @ w_v^T and v
    (natural layout, for the depthwise pe conv) by a second matmul -- no
    on-chip transposes anywhere.
  - AV out = (v^T)^T @ E accumulates over j in PSUM with 4 heads packed
    into one 128-partition tile via tile_position col groups; the softmax
    denominator accumulates in parallel via ones(128,32)^T @ E matmuls
    which also pre-broadcast d across each head's 32 output partitions.
    Both quads are emitted per exp-half so they start as soon as either
    engine finishes its half.
  - y = av * (1/d) + pe(v) on VectorE (reciprocal_approx_fast), then the
    proj matmul with bias added on ScalarE/VectorE.

The phase-2 loop is software-pipelined: each step emits the NEXT step's
score quad before this step's AV/d quads so the PE stream stays one step
ahead of the exp engines; all remaining projection work ("extras") is
drip-fed one chunk per step with producer-before-consumer deadlines
assert-checked.  On DVE-exp steps the extras' PSUM->SBUF bias copies run as
ScalarE Identity activations instead, balancing both engine queues at
~47us each.  PSUM budget (8 banks): 3 double-buffered S slots (6) + AV
accumulator (1) + denominator accumulator (1); phase-1/3 matmuls borrow S
slots.
"""

import os

import ml_dtypes
import numpy as np

import concourse.bass as bass
import concourse.mybir as mybir
import concourse.tile as tile
from concourse import bacc
from concourse.bass_utils import run_bass_kernel_spmd

BF16 = mybir.dt.bfloat16
F32 = mybir.dt.float32
I16 = mybir.dt.int16
AF = mybir.ActivationFunctionType
ALU = mybir.AluOpType

NH, KD, HD = 8, 16, 32
DIM, L, B = 256, 1024, 8
SCALE = KD ** -0.5  # 0.25

# Schraudolph exp-as-int16 constants: bf16(2^f) bit pattern via
# int16(round(f * 128 + 127*128)); scale folds SCALE*log2(e).
SCH_A = SCALE * float(np.log2(np.e)) * 128.0
SCH_B = 127.0 * 128.0 - 5.5  # -5.5: center the one-sided PL-interp error

# Steps whose p=1 exp tile runs on VectorE (Schraudolph). Excluded: the
# finish_tn steps 7/15/23 (DVE does reciprocal+multiply+pe-add there) and
# the peacc steps 20/25 (DVE does the depthwise conv), plus 1/4 whose DVE
# extras (vt+kq) already fill the slot.  Step 31 IS included: running the
# two final exps concurrently on both engines shortens the epilogue.
DVE_STEPS = frozenset(range(32)) - {1, 4, 7, 9, 15, 20, 23, 25}


def _install_ntff_shim():
    """Optionally register the axon NTFF profiling hook (for trace=True).

    The container's antenv package lacks axon_hooks; recreate it and wire the
    ctypes-based hook from trn_agent_boot so neuron-profile exec times work.
    """
    import sys
    import types

    name = "antenv.axon_hooks"
    if name in sys.modules:
        return
    try:
        import antenv
        from trn_agent_boot.trn_boot import _ntff_profile_via_ctypes
    except ImportError:
        return
    hooks = types.ModuleType(name)
    hooks._the_hook = None
    hooks.set_axon_ntff_profile_hook = lambda h: setattr(hooks, "_the_hook", h)
    hooks.get_axon_ntff_profile_hook = lambda: hooks._the_hook
    sys.modules[name] = hooks
    antenv.axon_hooks = hooks
    so = "/opt/axon/libaxon_pjrt.so"
    if os.path.exists(so):
        hook = _ntff_profile_via_ctypes(so)
        if hook is not None:
            hooks.set_axon_ntff_profile_hook(hook)


def build_kernel() -> bass.Bass:
    nc = bacc.Bacc("TRN2", target_bir_lowering=False, debug=False, num_devices=8)

    # ---- DRAM I/O (per-core shard; weights replicated) ----
    # All tensors per-partition contiguous (128 descriptors per DMA);
    # wt0 = [wk_t0 | wq_t0] is the minimal front set, wrest carries the
    # rest of the weights, pfb packs all small params + the b_v broadcast.
    xa_d = nc.dram_tensor("xa", (128, 2, 512), BF16, kind="ExternalInput")
    xb_d = nc.dram_tensor("xb", (128, 2, 512), BF16, kind="ExternalInput")
    wt0_d = nc.dram_tensor("wt0", (128, 2, 256), BF16, kind="ExternalInput")
    wrest_d = nc.dram_tensor("wrest", (128, 2, 768), BF16, kind="ExternalInput")
    # [bk(2) | bq(2) | bv(2) | wpe(6) | bpe(2) | bproj(2) | bvb(256)]
    pfb_d = nc.dram_tensor("pfb", (128, 272), F32, kind="ExternalInput")
    out_d = nc.dram_tensor("out", (128, 2, 1024), BF16, kind="ExternalOutput")

    with tile.TileContext(nc) as tc:
        with (
            tc.tile_pool(name="const", bufs=1) as cpool,
            tc.tile_pool(name="work", bufs=3) as wpool,
            tc.tile_pool(name="epool", bufs=10) as epool,
            tc.tile_pool(name="ps_s", bufs=3, space="PSUM") as ps_s,
            tc.tile_pool(name="ps_av", bufs=1, space="PSUM") as ps_av,
            tc.tile_pool(name="ps_d", bufs=1, space="PSUM") as ps_d,
        ):
            # phase-1/3 matmuls borrow the S pool's (128, 1024) slots so the
            # whole kernel fits in 8 PSUM banks: 3*2 (S) + 1 (av) + 1 (d).
            _misc_n = [0]

            def misc_ps(width=512):
                _misc_n[0] += 1
                return ps_s.tile(
                    [128, 1024], F32, tag="S", name=f"misc{_misc_n[0]}"
                )[:, :width]
            # ---- load constants / activations ----
            xa = cpool.tile([128, 2, 512], BF16, tag="xa")
            xb = cpool.tile([128, 2, 512], BF16, tag="xb")
            wt0 = cpool.tile([128, 2, 256], BF16, tag="wt0")
            wrest = cpool.tile([128, 2, 768], BF16, tag="wrest")
            pfb = cpool.tile([128, 272], F32, tag="pfb")
            onesb = cpool.tile([128, 32], BF16, tag="onesb")
            wv = wrest[:, :, 256:512]
            wpt = wrest[:, :, 512:768]
            pf = pfb[:, 0:16]
            bvb = pfb[:, 16:272]
            bk, bq, bv = pf[:, 0:2], pf[:, 2:4], pf[:, 4:6]
            wpe = pf[:, 6:12].rearrange("p (t k) -> p t k", t=2)
            bpe, bproj = pf[:, 12:14], pf[:, 14:16]

            def xh(n):
                return xa if n == 0 else xb

            def wkq(t):
                # [wk_t | wq_t]: t=0 in wt0, t=1 in wrest cols 0:256
                return wt0 if t == 0 else wrest

            # DMA order drives the critical path: the 16 rings round-robin
            # between the active queues at transfer granularity, so the
            # front set is interleaved I/X in need-order: wt0, xa-kc0, pfb,
            # xa-kc1, then xb; wrest (latest consumer) takes the gpsimd
            # SWDGE path, its descriptor push delayed behind a filler
            # memset so it cannot jump ahead of xa's second half.
            # The rings round-robin between queues at transfer granularity
            # (I first), so this interleaves as: wt0, xa-kc0, xa-kc1, pf
            # (tiny 8KB param block -- split from pfb so the front biases
            # aren't gated by the big bvb half), bvb, xb.
            nc.sync.dma_start(wt0[:], wt0_d.ap())
            nc.sync.dma_start(xa[:, 1, :], xa_d.ap()[:, 1, :])
            nc.scalar.dma_start(xa[:, 0, :], xa_d.ap()[:, 0, :])
            nc.scalar.dma_start(pfb[:, 0:16], pfb_d.ap()[:, 0:16])
            nc.scalar.dma_start(pfb[:, 16:272], pfb_d.ap()[:, 16:272])
            nc.scalar.dma_start(xb[:], xb_d.ap())
            # scratch memset first so the PE pre-warm burst can start early;
            # all on gpsimd whose queue is otherwise idle up front.
            scratch = cpool.tile([128, 512], BF16, tag="scratch")
            nc.gpsimd.memset(scratch[:], 0.0)
            nc.gpsimd.memset(onesb[:], 1.0)
            nc.gpsimd.dma_start(wrest[:], wrest_d.ap())

            # Pre-warm burst: the HAM clock gate needs >=3.4us of CONTINUOUS
            # matmul streaming before it unthrottles the PE from 1.2 to
            # 2.4 GHz (a shorter burst never trips the SHORT window and the
            # whole loop then runs at half clock).  9 x 427ns cold matmuls
            # guarantee the flip while the input DMAs land.
            warm_ps = misc_ps()
            for _ in range(9):
                nc.tensor.matmul(
                    warm_ps[:], scratch[:, :128], scratch[:],
                    start=True, stop=True, skip_group_check=True,
                )

            # persistent intermediates
            tk = cpool.tile([128, 2, 1024], BF16, tag="tk")        # packed k
            tq = cpool.tile([128, 2, 1024], BF16, tag="tq")        # packed q
            vnat = cpool.tile([128, 2, 1024], BF16, tag="vnat")    # v, natural
            vT = cpool.tile([128, 8, 256], BF16, tag="vT")         # v^T, j-chunked
            peacc = cpool.tile([128, 2, 1024], BF16, tag="peacc")  # pe conv terms
            ybf = cpool.tile([128, 2, 1024], BF16, tag="ybf")      # y = av*R + pe
            zout = cpool.tile([128, 2, 1024], BF16, tag="zout")

            # ---- phase-1 building blocks (emitted piecemeal) ----
            kq_ready = set()
            vt_ready = set()
            vn_ready = set()

            def bias_copy(dst, src, b_ap, on_scalar):
                # PSUM -> SBUF with per-partition bias: VectorE tensor_scalar
                # or (on DVE-exp steps) ScalarE Identity activation.
                if on_scalar:
                    nc.scalar.activation(dst, src, AF.Identity, bias=b_ap)
                else:
                    nc.vector.tensor_scalar(dst, src, b_ap, None, ALU.add)

            def emit_kq(t, n, which, on_scalar=False):
                off, b_sb, dst = (0, bk, tk) if which == "k" else (128, bq, tq)
                kq_ready.add((id(dst), t, n))
                ps = misc_ps()
                for kc in range(2):
                    nc.tensor.matmul(
                        ps[:], wkq(t)[:, kc, off:off + 128],
                        xh(n)[:, kc, :],
                        start=(kc == 0), stop=(kc == 1),
                    )
                bias_copy(
                    dst[:, t, n * 512:(n + 1) * 512], ps[:],
                    b_sb[:, t:t + 1], on_scalar,
                )

            def emit_vnat(t, n, on_scalar=False):
                vn_ready.add((t, n))
                ps = misc_ps()
                for kc in range(2):
                    nc.tensor.matmul(
                        ps[:], wv[:, kc, t * 128:(t + 1) * 128],
                        xh(n)[:, kc, :],
                        start=(kc == 0), stop=(kc == 1),
                    )
                bias_copy(
                    vnat[:, t, n * 512:(n + 1) * 512], ps[:],
                    bv[:, t:t + 1], on_scalar,
                )

            def emit_vt(jc):
                vt_ready.add(jc)
                ps = misc_ps()
                for kc in range(2):
                    nc.tensor.matmul(
                        ps[:, :256],
                        xh(jc // 4)[:, kc, (jc % 4) * 128:(jc % 4 + 1) * 128],
                        wv[:, kc, :],
                        start=(kc == 0), stop=(kc == 1),
                    )
                nc.vector.tensor_tensor(vT[:, jc, :], ps[:, :256], bvb[:], ALU.add)

            # pe = depthwise conv(k=3, pad 1) on v + bias, into peacc[:, t, :]
            peacc_done = [False, False]
            pe_pending = []

            def emit_peacc(t):
                assert (t, 0) in vn_ready and (t, 1) in vn_ready
                nc.vector.tensor_scalar(
                    peacc[:, t, :], vnat[:, t, :], wpe[:, t, 1:2], bpe[:, t:t + 1],
                    ALU.mult, ALU.add,
                )
                tmp_l = wpool.tile([128, 1024], BF16, tag="pel", name=f"pel{t}")
                nc.vector.tensor_scalar(
                    tmp_l[:, :1023], vnat[:, t, :1023], wpe[:, t, 0:1], None,
                    ALU.mult,
                )
                nc.vector.tensor_tensor(
                    peacc[:, t, 1:], peacc[:, t, 1:], tmp_l[:, :1023], ALU.add,
                )
                tmp_r = wpool.tile([128, 1024], BF16, tag="per", name=f"per{t}")
                nc.vector.tensor_scalar(
                    tmp_r[:, :1023], vnat[:, t, 1:], wpe[:, t, 2:3], None,
                    ALU.mult,
                )
                nc.vector.tensor_tensor(
                    peacc[:, t, :1023], peacc[:, t, :1023], tmp_r[:, :1023], ALU.add,
                )
                peacc_done[t] = True
                for (tt_, nn_) in [p for p in pe_pending if p[0] == t]:
                    pe_pending.remove((tt_, nn_))
                    emit_pe_add(tt_, nn_)

            def emit_pe_add(t, n):
                # SBUF->SBUF bf16 add: runs on the otherwise-idle GpSimd
                # engine so the loaded VectorE queue isn't on this path.
                nc.gpsimd.tensor_tensor(
                    ybf[:, t, n * 512:(n + 1) * 512],
                    ybf[:, t, n * 512:(n + 1) * 512],
                    peacc[:, t, n * 512:(n + 1) * 512], ALU.add,
                )

            def emit_proj(mo, n, ps, bias_engine=None):
                for kc in range(2):
                    nc.tensor.matmul(
                        ps[:], wpt[:, kc, mo * 128:(mo + 1) * 128],
                        ybf[:, kc, n * 512:(n + 1) * 512],
                        start=(kc == 0), stop=(kc == 1),
                    )
                if bias_engine is nc.scalar:
                    nc.scalar.activation(
                        zout[:, mo, n * 512:(n + 1) * 512], ps[:],
                        AF.Identity, bias=bproj[:, mo:mo + 1],
                    )
                else:
                    nc.vector.tensor_scalar(
                        zout[:, mo, n * 512:(n + 1) * 512], ps[:],
                        bproj[:, mo:mo + 1], None, ALU.add,
                    )
                nc.sync.dma_start(
                    out_d.ap()[:, mo, n * 512:(n + 1) * 512],
                    zout[:, mo, n * 512:(n + 1) * 512],
                )

            # ---- phase 2: software-pipelined attention ----
            # steps (t, n, jc): per step one S quad / two exps / AV + d quads.
            # Phase-1 work not needed up front is drip-fed one chunk per step
            # ("extras") so the exp chain starts as early as possible.
            steps = [
                (t, n, jc)
                for t in range(2) for n in range(2) for jc in range(8)
            ]
            av_tiles_all = {}
            d_tiles_all = {}

            def emit_s_half(step, p):
                # 2 score matmuls at distinct row groups -> run concurrently.
                t, n, jc = step
                assert (id(tk), t, 0) in kq_ready and (id(tq), t, n) in kq_ready
                assert jc < 4 or (id(tk), t, 1) in kq_ready
                s_ps = ps_s.tile(
                    [128, 1024], F32, tag="S", name=f"s_{t}_{n}_{jc}_{p}"
                )
                for gg in range(2):
                    g = 2 * p + gg
                    nc.tensor.matmul(
                        s_ps[:, gg * 512:(gg + 1) * 512],
                        tk[32 * g:32 * g + 16, t, jc * 128:(jc + 1) * 128],
                        tq[32 * g:32 * g + 16, t, n * 512:(n + 1) * 512],
                        start=True, stop=True,
                        tile_position=(32 * g, 0),
                    )
                return s_ps

            def finish_tn(t, n):
                # R = 1/d (d ~ L, far from reciprocal_approx edge cases)
                av_ps = av_tiles_all.pop((t, n))
                d_ps = d_tiles_all.pop((t, n))
                rb_sb = wpool.tile([128, 512], F32, tag="rb")
                nc.vector.reciprocal_approx_fast(rb_sb[:], d_ps[:])
                nc.vector.tensor_tensor(
                    ybf[:, t, n * 512:(n + 1) * 512], av_ps[:], rb_sb[:],
                    ALU.mult,
                )
                if peacc_done[t]:
                    emit_pe_add(t, n)
                else:
                    pe_pending.append((t, n))

            # minimal front for step (0, 0, 0): the first S quad only reads
            # tk columns 0:128 (j-chunk 0), so produce just those first (a
            # cheap N=128 matmul), then full tq; the rest of tk's n=0 half
            # follows behind the S quad.
            # tq first (the longer matmul chain) with its two kc matmuls
            # split so kc=0 streams while xa's kc=1 half is still landing;
            # bias on the idle ScalarE (table already loaded); tk0a bias on
            # VectorE so both biases run concurrently.
            kq_ready.add((id(tq), 0, 0))
            ps = misc_ps()
            for kc in range(2):
                nc.tensor.matmul(
                    ps[:], wkq(0)[:, kc, 128:256], xa[:, kc, :],
                    start=(kc == 0), stop=(kc == 1),
                )
            nc.scalar.activation(
                tq[:, 0, :512], ps[:], AF.Identity, bias=bq[:, 0:1],
            )
            kq_ready.add((id(tk), 0, 0))
            ps = misc_ps()
            for kc in range(2):
                nc.tensor.matmul(
                    ps[:, :128], wkq(0)[:, kc, :128], xa[:, kc, :128],
                    start=(kc == 0), stop=(kc == 1),
                )
            nc.vector.tensor_scalar(
                tk[:, 0, :128], ps[:, :128], bk[:, 0:1], None, ALU.add,
            )
            # drip-fed producers; ORDER MATTERS: vT[jc] must be emitted no
            # later than step jc (its AV consumer), tk/tq halves before the
            # S quads that read them (S for step i+1 is emitted during i).
            # Extras marked "sc" run their PSUM->SBUF bias copy on ScalarE --
            # only placed on DVE_STEPS where ScalarE has a free slot.
            extras = {
                0: [lambda: emit_vt(0)],
                1: [lambda: emit_vt(1), lambda: emit_kq(0, 1, "k")],
                2: [lambda: emit_vt(2)],
                3: [lambda: emit_vt(3)],
                4: [lambda: emit_vt(4)],
                5: [lambda: emit_vt(5), lambda: emit_kq(0, 1, "q", True)],
                6: [lambda: emit_vt(6)],
                7: [lambda: emit_vt(7)],
                9: [lambda: emit_kq(1, 0, "k")],
                10: [lambda: emit_kq(1, 0, "q", True)],
                12: [lambda: emit_kq(1, 1, "k", True)],
                14: [lambda: emit_kq(1, 1, "q", True)],
                16: [lambda: emit_vnat(0, 0)],
                18: [lambda: emit_vnat(0, 1)],
                20: [lambda: emit_peacc(0)],
                21: [lambda: emit_vnat(1, 0)],
                24: [lambda: emit_vnat(1, 1)],
                25: [lambda: emit_peacc(1)],
                # n=0 proj units: both n=0 halves of y are ready after step
                # 23; borrow an S slot, bias on ScalarE, so the whole n=0
                # output (matmul + bias + store) completes mid-stream.
                26: [lambda: emit_proj(0, 0, misc_ps(), nc.scalar)],
                27: [lambda: emit_proj(1, 0, misc_ps(), nc.scalar)],
            }

            s_next = [emit_s_half(steps[0], 0), emit_s_half(steps[0], 1)]
            # tk n=0 columns 128:512 (j-chunks 1-3; their S quads are emitted
            # from step 0 onward, always after this point in the PE stream)
            ps = misc_ps()
            for kc in range(2):
                nc.tensor.matmul(
                    ps[:, :384], wkq(0)[:, kc, :128], xa[:, kc, 128:512],
                    start=(kc == 0), stop=(kc == 1),
                )
            nc.vector.tensor_scalar(
                tk[:, 0, 128:512], ps[:, :384], bk[:, 0:1], None, ALU.add,
            )
            for i, step in enumerate(steps):
                t, n, jc = step
                s_cur = s_next
                use_dve = i in DVE_STEPS
                e_sb = []
                for p in range(2):
                    e = epool.tile([128, 1024], BF16, tag="E", name=f"e{i}_{p}")
                    if p == 1 and use_dve:
                        nc.vector.tensor_scalar(
                            e[:].bitcast(I16), s_cur[p][:],
                            SCH_A, SCH_B, ALU.mult, ALU.add,
                        )
                    else:
                        nc.scalar.activation(e[:], s_cur[p][:], AF.Exp, scale=SCALE)
                    e_sb.append(e)
                # extras first: they have no exp dependencies, so the PE
                # runs them in the window while this step's exps are still
                # in flight.
                for fn in extras.pop(i, []):
                    fn()
                if (t, n) not in av_tiles_all:
                    av_tiles_all[(t, n)] = ps_av.tile(
                        [128, 512], F32, tag="av", name=f"av_{t}_{n}"
                    )
                    d_tiles_all[(t, n)] = ps_d.tile(
                        [128, 512], F32, tag="d", name=f"d_{t}_{n}"
                    )
                av_ps = av_tiles_all[(t, n)]
                d_ps = d_tiles_all[(t, n)]
                assert jc in vt_ready, (t, n, jc)

                def quads(gs):
                    for g in gs:
                        # denominator, pre-broadcast: ones(128,32)^T @ E
                        # fills all 32 partitions of head g with d_h[i].
                        # Before the AV matmuls so the tail's reciprocal
                        # starts earlier.
                        nc.tensor.matmul(
                            d_ps[32 * g:32 * g + 32, :],
                            onesb[:, :32],
                            e_sb[g // 2][:, (g % 2) * 512:(g % 2 + 1) * 512],
                            start=(jc == 0), stop=(jc == 7),
                            tile_position=(0, 32 * g),
                            skip_group_check=True,
                        )
                    for g in gs:
                        h = 4 * t + g
                        nc.tensor.matmul(
                            av_ps[32 * g:32 * g + 32, :],
                            vT[:, jc, 32 * h:32 * h + 32],
                            e_sb[g // 2][:, (g % 2) * 512:(g % 2 + 1) * 512],
                            start=(jc == 0), stop=(jc == 7),
                            tile_position=(0, 32 * g),
                            skip_group_check=True,
                        )

                # PE emission order per step type, chosen so the PE stream
                # packs FIFO-tight behind each exp's completion:
                #  - DVE steps (p1 exp ends early on VectorE): S halves for
                #    the next step, then full-width quads (4-way column
                #    concurrency).
                #  - ScalarE-only steps (p1 exp ends a full EXP later): the
                #    p0-dependent quad half fills the gap before S-p1.
                nxt = steps[i + 1] if i + 1 < len(steps) else None
                if nxt is None:
                    quads([0, 1])
                    quads([2, 3])
                elif use_dve:
                    s_next = [emit_s_half(nxt, 0), emit_s_half(nxt, 1)]
                    quads([0, 1, 2, 3])
                else:
                    sA = emit_s_half(nxt, 0)
                    quads([0, 1])
                    s_next = [sA, emit_s_half(nxt, 1)]
                    quads([2, 3])
                # HAM duty filler: one throwaway matmul accumulating +0.0
                # (zero weights, start=False) into the open d accumulator.
                # It has NO cross-engine dependencies -- pure PE-FIFO work at
                # the tail of the step's burst -- so it raises the array's
                # duty cycle (keeping the HAM clock gate at 2.4 GHz, which
                # otherwise oscillates and halves every matmul) without
                # serializing the exp chains.
                if jc < 7:
                    nc.tensor.matmul(
                        d_ps[0:32, :384], scratch[:, :32], xa[:, 0, :384],
                        start=False, stop=False,
                        tile_position=(0, 0),
                        skip_group_check=True,
                    )
                if jc == 7:
                    finish_tn(t, n)
            assert not extras

            # ---- phase 3: remaining (n=1) proj units ----
            # kc=0 (reads the long-finished t=0 half of y) is emitted first so
            # it executes off the critical tail chain; kc=1 + bias + DMA wait
            # only on the last pe add.
            prj_ps = []
            for mo in range(2):
                ps = misc_ps()
                prj_ps.append(ps)
                nc.tensor.matmul(
                    ps[:], wpt[:, 0, mo * 128:(mo + 1) * 128],
                    ybf[:, 0, 512:], start=True, stop=False,
                )
            for mo in range(2):
                ps = prj_ps[mo]
                nc.tensor.matmul(
                    ps[:], wpt[:, 1, mo * 128:(mo + 1) * 128],
                    ybf[:, 1, 512:], start=False, stop=True,
                )
                # biases on different engines, stores from both HWDGE
                # queues, so the two final units finish in parallel
                if mo == 0:
                    nc.scalar.activation(
                        zout[:, mo, 512:], ps[:],
                        AF.Identity, bias=bproj[:, mo:mo + 1],
                    )
                else:
                    nc.vector.tensor_scalar(
                        zout[:, mo, 512:], ps[:],
                        bproj[:, mo:mo + 1], None, ALU.add,
                    )
                dma_q = nc.sync if mo == 0 else nc.scalar
                dma_q.dma_start(
                    out_d.ap()[:, mo, 512:], zout[:, mo, 512:],
                )

    nc.compile()
    return nc


def pack_inputs(x, w_qkv, b_qkv, w_pe, b_pe, w_proj, b_proj):
    """Host-side packing of the full inputs into per-core in_maps."""
    bf16 = ml_dtypes.bfloat16
    f32 = np.float32

    # k/q packed layouts: tile t in {0,1}; partition m = 32*g + r; head h = 4t+g.
    # Only r < 16 is live (k channel r -> qkv row 64h+16+r; q channel r -> 64h+r);
    # r >= 16 columns are zero so both tiles stay 32-aligned per head.
    w_kA = np.zeros((256, 256), dtype=w_qkv.dtype)
    w_qA = np.zeros((256, 256), dtype=w_qkv.dtype)
    b_kP = np.zeros((128, 2), dtype=b_qkv.dtype)
    b_qP = np.zeros((128, 2), dtype=b_qkv.dtype)
    for t in range(2):
        for m in range(128):
            g, r = m // 32, m % 32
            h = 4 * t + g
            if r < 16:
                w_kA[:, t * 128 + m] = w_qkv[64 * h + 16 + r]
                w_qA[:, t * 128 + m] = w_qkv[64 * h + r]
                b_kP[m, t] = b_qkv[64 * h + 16 + r]
                b_qP[m, t] = b_qkv[64 * h + r]

    v_rows = np.array([64 * (c // 32) + 32 + c % 32 for c in range(256)])
    w_v = w_qkv[v_rows].T  # (256 d, 256 c)
    b_v = b_qkv[v_rows]

    def kpart(a):  # (256, F) -> (128, 2, F)
        return np.ascontiguousarray(a.reshape(2, 128, -1).transpose(1, 0, 2))

    def chan2(a):  # (256,) -> (128, 2)
        return np.ascontiguousarray(a.reshape(2, 128).T)

    pf = np.concatenate([
        b_kP, b_qP, chan2(b_v),
        kpart(w_pe[:, 0, :]).reshape(128, 6),
        chan2(b_pe), chan2(b_proj),
    ], axis=1).astype(f32)  # (128, 16)
    bvb = np.broadcast_to(b_v[None, :], (128, 256)).astype(f32)
    pfb = np.ascontiguousarray(np.concatenate([pf, bvb], axis=1))  # (128, 272)

    kA = kpart(w_kA)   # (128, 2, 256): [:, :, t*128:(t+1)*128] is tile t
    qA = kpart(w_qA)
    vA = kpart(w_v)
    pA = kpart(w_proj.T)
    wt0 = np.concatenate([kA[:, :, 0:128], qA[:, :, 0:128]], axis=2)
    wrest = np.concatenate(
        [kA[:, :, 128:256], qA[:, :, 128:256], vA, pA], axis=2
    )
    common = {
        "wt0": np.ascontiguousarray(wt0).astype(bf16),
        "wrest": np.ascontiguousarray(wrest).astype(bf16),
        "pfb": pfb,
    }
    in_maps = []
    for b in range(B):
        m = dict(common)
        xp = kpart(x[b]).astype(bf16)
        m["xa"] = np.ascontiguousarray(xp[:, :, :512])
        m["xb"] = np.ascontiguousarray(xp[:, :, 512:])
        in_maps.append(m)
    return in_maps


_CACHE = {}


def kernel(x, w_qkv, b_qkv, w_pe, b_pe, w_proj, b_proj):
    x = np.asarray(x, dtype=np.float32)
    w_qkv = np.asarray(w_qkv, dtype=np.float32)
    b_qkv = np.asarray(b_qkv, dtype=np.float32)
    w_pe = np.asarray(w_pe, dtype=np.float32)
    b_pe = np.asarray(b_pe, dtype=np.float32)
    w_proj = np.asarray(w_proj, dtype=np.float32)
    b_proj = np.asarray(b_proj, dtype=np.float32)

    if "nc" not in _CACHE:
        _CACHE["nc"] = build_kernel()
    nc = _CACHE["nc"]

    in_maps = pack_inputs(x, w_qkv, b_qkv, w_pe, b_pe, w_proj, b_proj)

    trace = os.environ.get("BASS_KERNEL_TRACE", "") == "1"
    if trace:
        _install_ntff_shim()
    res = run_bass_kernel_spmd(
        nc, in_maps, core_ids=list(range(B)), trace=trace,
    )
    if trace:
        _CACHE["last_result"] = res

    out = np.empty((B, DIM, L), dtype=np.float32)
    for b in range(B):
        z = np.asarray(res.results[b]["out"], dtype=np.float32)  # (128, 2, 1024)
        out[b] = z.transpose(1, 0, 2).reshape(DIM, L)
    return out


# revision 20
# speedup vs baseline: 1.0242x; 1.0242x over previous
"""Trainium2 Bass kernel for nn_Attention_13778255085887.

Dense multi-head attention block (EfficientViT-style):
  qkv 1x1 conv -> per-head softmax(q^T k * scale) -> v @ attn^T
  + depthwise conv(k=3) positional encoding on v -> proj 1x1 conv.

Shapes: B=8, dim=256, L=1024, heads=8, key_dim=16, head_dim=32.

Strategy: data-parallel over B across the 8 NeuronCores (zero collectives).
The serial bottleneck is the 8.4M-element softmax exp per core.  It is split
across TWO engines running concurrently:

  - ScalarE runs exact ACTIVATE-Exp on ~60% of the (128, 1024) score tiles.
  - VectorE runs a Schraudolph-style exp on the rest: one fused
    tensor_scalar  int16(round(x * 0.25*log2e*128 + (16256 - 5.5)))  whose
    int16 bit pattern IS the bf16 encoding of 2^(0.25*x*log2e) ~ exp(x/4)
    (max ~3% elementwise error; softmax weights are near-uniform at this
    scale so the error averages out ~30x below the 2e-2 gate -- verified in
    numpy simulation end-to-end at ~7e-3 vs 3.6e-3 all-exact).

  - q/k are projected into a packed layout (4 heads per 128-partition tile,
    head g at partitions 32g..32g+16, zero-padded to 32) so the tiny K=16
    score matmuls run 4-at-a-time via tile_position row groups.
  - S^T = k^T q is built per (head-pair, j-chunk) directly with j on
    partitions; softmax-without-max (logits bounded ~[-1.5, 1.5]) makes
    exp a single instruction per (128, 1024) PSUM block.
  - v^T (AV stationary operand) is computed directly as x^T @ w_v^T and v
    (natural layout, for the depthwise pe conv) by a second matmul -- no
    on-chip transposes anywhere.
  - AV out = (v^T)^T @ E accumulates over j in PSUM with 4 heads packed
    into one 128-partition tile via tile_position col groups; the softmax
    denominator accumulates in parallel via ones(128,32)^T @ E matmuls
    which also pre-broadcast d across each head's 32 output partitions.
    Both quads are emitted per exp-half so they start as soon as either
    engine finishes its half.
  - y = av * (1/d) + pe(v) on VectorE (reciprocal_approx_fast), then the
    proj matmul with bias added on ScalarE/VectorE.

The phase-2 loop is software-pipelined: each step emits the NEXT step's
score quad before this step's AV/d quads so the PE stream stays one step
ahead of the exp engines; all remaining projection work ("extras") is
drip-fed one chunk per step with producer-before-consumer deadlines
assert-checked.  On DVE-exp steps the extras' PSUM->SBUF bias copies run as
ScalarE Identity activations instead, balancing both engine queues at
~47us each.  PSUM budget (8 banks): 3 double-buffered S slots (6) + AV
accumulator (1) + denominator accumulator (1); phase-1/3 matmuls borrow S
slots.
"""

import os

import ml_dtypes
import numpy as np

import concourse.bass as bass
import concourse.mybir as mybir
import concourse.tile as tile
from concourse import bacc
from concourse.bass_utils import run_bass_kernel_spmd

BF16 = mybir.dt.bfloat16
F32 = mybir.dt.float32
I16 = mybir.dt.int16
AF = mybir.ActivationFunctionType
ALU = mybir.AluOpType

NH, KD, HD = 8, 16, 32
DIM, L, B = 256, 1024, 8
SCALE = KD ** -0.5  # 0.25

# Schraudolph exp-as-int16 constants: bf16(2^f) bit pattern via
# int16(round(f * 128 + 127*128)); scale folds SCALE*log2(e).
SCH_A = SCALE * float(np.log2(np.e)) * 128.0
SCH_B = 127.0 * 128.0 - 5.5  # -5.5: center the one-sided PL-interp error

# Steps whose p=1 exp tile runs on VectorE (Schraudolph). Excluded: the
# finish_tn steps 7/15/23 (DVE does reciprocal+multiply+pe-add there) and
# the peacc steps 20/25 (DVE does the depthwise conv), plus 1/4 whose DVE
# extras (vt+kq) already fill the slot.  Step 31 IS included: running the
# two final exps concurrently on both engines shortens the epilogue.
DVE_STEPS = frozenset(range(32)) - {1, 4, 7, 9, 15, 20, 23, 25}


def _install_ntff_shim():
    """Optionally register the axon NTFF profiling hook (for trace=True).

    The container's antenv package lacks axon_hooks; recreate it and wire the
    ctypes-based hook from trn_agent_boot so neuron-profile exec times work.
    """
    import sys
    import types

    name = "antenv.axon_hooks"
    if name in sys.modules:
        return
    try:
        import antenv
        from trn_agent_boot.trn_boot import _ntff_profile_via_ctypes
    except ImportError:
        return
    hooks = types.ModuleType(name)
    hooks._the_hook = None
    hooks.set_axon_ntff_profile_hook = lambda h: setattr(hooks, "_the_hook", h)
    hooks.get_axon_ntff_profile_hook = lambda: hooks._the_hook
    sys.modules[name] = hooks
    antenv.axon_hooks = hooks
    so = "/opt/axon/libaxon_pjrt.so"
    if os.path.exists(so):
        hook = _ntff_profile_via_ctypes(so)
        if hook is not None:
            hooks.set_axon_ntff_profile_hook(hook)


def build_kernel() -> bass.Bass:
    nc = bacc.Bacc("TRN2", target_bir_lowering=False, debug=False, num_devices=8)

    # ---- DRAM I/O (per-core shard; weights replicated) ----
    # All tensors per-partition contiguous (128 descriptors per DMA);
    # wt0 = [wk_t0 | wq_t0] is the minimal front set, wrest carries the
    # rest of the weights, pfb packs all small params + the b_v broadcast.
    xa_d = nc.dram_tensor("xa", (128, 2, 512), BF16, kind="ExternalInput")
    xb_d = nc.dram_tensor("xb", (128, 2, 512), BF16, kind="ExternalInput")
    wt0_d = nc.dram_tensor("wt0", (128, 2, 256), BF16, kind="ExternalInput")
    wrest_d = nc.dram_tensor("wrest", (128, 2, 768), BF16, kind="ExternalInput")
    # [bk(2) | bq(2) | bv(2) | wpe(6) | bpe(2) | bproj(2) | bvb(256)]
    pfb_d = nc.dram_tensor("pfb", (128, 272), F32, kind="ExternalInput")
    out_d = nc.dram_tensor("out", (128, 2, 1024), BF16, kind="ExternalOutput")

    with tile.TileContext(nc) as tc:
        with (
            tc.tile_pool(name="const", bufs=1) as cpool,
            tc.tile_pool(name="work", bufs=3) as wpool,
            tc.tile_pool(name="epool", bufs=10) as epool,
            tc.tile_pool(name="ps_s", bufs=3, space="PSUM") as ps_s,
            tc.tile_pool(name="ps_av", bufs=1, space="PSUM") as ps_av,
            tc.tile_pool(name="ps_d", bufs=1, space="PSUM") as ps_d,
        ):
            # phase-1/3 matmuls borrow the S pool's (128, 1024) slots so the
            # whole kernel fits in 8 PSUM banks: 3*2 (S) + 1 (av) + 1 (d).
            _misc_n = [0]

            def misc_ps(width=512):
                _misc_n[0] += 1
                return ps_s.tile(
                    [128, 1024], F32, tag="S", name=f"misc{_misc_n[0]}"
                )[:, :width]
            # ---- load constants / activations ----
            xa = cpool.tile([128, 2, 512], BF16, tag="xa")
            xb = cpool.tile([128, 2, 512], BF16, tag="xb")
            wt0 = cpool.tile([128, 2, 256], BF16, tag="wt0")
            wrest = cpool.tile([128, 2, 768], BF16, tag="wrest")
            pfb = cpool.tile([128, 272], F32, tag="pfb")
            onesb = cpool.tile([128, 32], BF16, tag="onesb")
            wv = wrest[:, :, 256:512]
            wpt = wrest[:, :, 512:768]
            pf = pfb[:, 0:16]
            bvb = pfb[:, 16:272]
            bk, bq, bv = pf[:, 0:2], pf[:, 2:4], pf[:, 4:6]
            wpe = pf[:, 6:12].rearrange("p (t k) -> p t k", t=2)
            bpe, bproj = pf[:, 12:14], pf[:, 14:16]

            def xh(n):
                return xa if n == 0 else xb

            def wkq(t):
                # [wk_t | wq_t]: t=0 in wt0, t=1 in wrest cols 0:256
                return wt0 if t == 0 else wrest

            # DMA order drives the critical path: the 16 rings round-robin
            # between the active queues at transfer granularity, so the
            # front set is interleaved I/X in need-order: wt0, xa-kc0, pfb,
            # xa-kc1, then xb; wrest (latest consumer) takes the gpsimd
            # SWDGE path, its descriptor push delayed behind a filler
            # memset so it cannot jump ahead of xa's second half.
            # The rings round-robin between queues at transfer granularity
            # (I first), so this interleaves as: wt0, xa-kc0, xa-kc1, pf
            # (tiny 8KB param block -- split from pfb so the front biases
            # aren't gated by the big bvb half), bvb, xb.
            nc.sync.dma_start(wt0[:], wt0_d.ap())
            nc.sync.dma_start(xa[:, 1, :], xa_d.ap()[:, 1, :])
            nc.scalar.dma_start(xa[:, 0, :], xa_d.ap()[:, 0, :])
            nc.scalar.dma_start(pfb[:, 0:16], pfb_d.ap()[:, 0:16])
            nc.scalar.dma_start(pfb[:, 16:272], pfb_d.ap()[:, 16:272])
            nc.scalar.dma_start(xb[:], xb_d.ap())
            # scratch memset first so the PE pre-warm burst can start early;
            # all on gpsimd whose queue is otherwise idle up front.
            scratch = cpool.tile([128, 512], BF16, tag="scratch")
            nc.gpsimd.memset(scratch[:], 0.0)
            nc.gpsimd.memset(onesb[:], 1.0)
            nc.gpsimd.dma_start(wrest[:], wrest_d.ap())

            # Pre-warm burst: the HAM clock gate needs >=3.4us of CONTINUOUS
            # matmul streaming before it unthrottles the PE from 1.2 to
            # 2.4 GHz (a shorter burst never trips the SHORT window and the
            # whole loop then runs at half clock).  9 x 427ns cold matmuls
            # guarantee the flip while the input DMAs land.
            warm_ps = misc_ps()
            for _ in range(9):
                nc.tensor.matmul(
                    warm_ps[:], scratch[:, :128], scratch[:],
                    start=True, stop=True, skip_group_check=True,
                )

            # persistent intermediates
            tk = cpool.tile([128, 2, 1024], BF16, tag="tk")        # packed k
            tq = cpool.tile([128, 2, 1024], BF16, tag="tq")        # packed q
            vnat = cpool.tile([128, 2, 1024], BF16, tag="vnat")    # v, natural
            vT = cpool.tile([128, 8, 256], BF16, tag="vT")         # v^T, j-chunked
            peacc = cpool.tile([128, 2, 1024], BF16, tag="peacc")  # pe conv terms
            ybf = cpool.tile([128, 2, 1024], BF16, tag="ybf")      # y = av*R + pe
            zout = cpool.tile([128, 2, 1024], BF16, tag="zout")

            # ---- phase-1 building blocks (emitted piecemeal) ----
            kq_ready = set()
            vt_ready = set()
            vn_ready = set()

            def bias_copy(dst, src, b_ap, on_scalar):
                # PSUM -> SBUF with per-partition bias: VectorE tensor_scalar
                # or (on DVE-exp steps) ScalarE Identity activation.
                if on_scalar:
                    nc.scalar.activation(dst, src, AF.Identity, bias=b_ap)
                else:
                    nc.vector.tensor_scalar(dst, src, b_ap, None, ALU.add)

            def emit_kq(t, n, which, on_scalar=False):
                off, b_sb, dst = (0, bk, tk) if which == "k" else (128, bq, tq)
                kq_ready.add((id(dst), t, n))
                ps = misc_ps()
                for kc in range(2):
                    nc.tensor.matmul(
                        ps[:], wkq(t)[:, kc, off:off + 128],
                        xh(n)[:, kc, :],
                        start=(kc == 0), stop=(kc == 1),
                    )
                bias_copy(
                    dst[:, t, n * 512:(n + 1) * 512], ps[:],
                    b_sb[:, t:t + 1], on_scalar,
                )

            def emit_vnat(t, n, on_scalar=False):
                vn_ready.add((t, n))
                ps = misc_ps()
                for kc in range(2):
                    nc.tensor.matmul(
                        ps[:], wv[:, kc, t * 128:(t + 1) * 128],
                        xh(n)[:, kc, :],
                        start=(kc == 0), stop=(kc == 1),
                    )
                bias_copy(
                    vnat[:, t, n * 512:(n + 1) * 512], ps[:],
                    bv[:, t:t + 1], on_scalar,
                )

            def emit_vt(jc):
                vt_ready.add(jc)
                ps = misc_ps()
                for kc in range(2):
                    nc.tensor.matmul(
                        ps[:, :256],
                        xh(jc // 4)[:, kc, (jc % 4) * 128:(jc % 4 + 1) * 128],
                        wv[:, kc, :],
                        start=(kc == 0), stop=(kc == 1),
                    )
                nc.vector.tensor_tensor(vT[:, jc, :], ps[:, :256], bvb[:], ALU.add)

            # pe = depthwise conv(k=3, pad 1) on v + bias, into peacc[:, t, :]
            peacc_done = [False, False]
            pe_pending = []

            def emit_peacc(t):
                assert (t, 0) in vn_ready and (t, 1) in vn_ready
                nc.vector.tensor_scalar(
                    peacc[:, t, :], vnat[:, t, :], wpe[:, t, 1:2], bpe[:, t:t + 1],
                    ALU.mult, ALU.add,
                )
                tmp_l = wpool.tile([128, 1024], BF16, tag="pel", name=f"pel{t}")
                nc.vector.tensor_scalar(
                    tmp_l[:, :1023], vnat[:, t, :1023], wpe[:, t, 0:1], None,
                    ALU.mult,
                )
                nc.vector.tensor_tensor(
                    peacc[:, t, 1:], peacc[:, t, 1:], tmp_l[:, :1023], ALU.add,
                )
                tmp_r = wpool.tile([128, 1024], BF16, tag="per", name=f"per{t}")
                nc.vector.tensor_scalar(
                    tmp_r[:, :1023], vnat[:, t, 1:], wpe[:, t, 2:3], None,
                    ALU.mult,
                )
                nc.vector.tensor_tensor(
                    peacc[:, t, :1023], peacc[:, t, :1023], tmp_r[:, :1023], ALU.add,
                )
                peacc_done[t] = True
                for (tt_, nn_) in [p for p in pe_pending if p[0] == t]:
                    pe_pending.remove((tt_, nn_))
                    emit_pe_add(tt_, nn_)

            def emit_pe_add(t, n):
                # SBUF->SBUF bf16 add: runs on the otherwise-idle GpSimd
                # engine so the loaded VectorE queue isn't on this path.
                nc.gpsimd.tensor_tensor(
                    ybf[:, t, n * 512:(n + 1) * 512],
                    ybf[:, t, n * 512:(n + 1) * 512],
                    peacc[:, t, n * 512:(n + 1) * 512], ALU.add,
                )

            def emit_proj(mo, n, ps, bias_engine=None):
                for kc in range(2):
                    nc.tensor.matmul(
                        ps[:], wpt[:, kc, mo * 128:(mo + 1) * 128],
                        ybf[:, kc, n * 512:(n + 1) * 512],
                        start=(kc == 0), stop=(kc == 1),
                    )
                if bias_engine is nc.scalar:
                    nc.scalar.activation(
                        zout[:, mo, n * 512:(n + 1) * 512], ps[:],
                        AF.Identity, bias=bproj[:, mo:mo + 1],
                    )
                else:
                    nc.vector.tensor_scalar(
                        zout[:, mo, n * 512:(n + 1) * 512], ps[:],
                        bproj[:, mo:mo + 1], None, ALU.add,
                    )
                nc.sync.dma_start(
                    out_d.ap()[:, mo, n * 512:(n + 1) * 512],
                    zout[:, mo, n * 512:(n + 1) * 512],
                )

            # ---- phase 2: software-pipelined attention ----
            # steps (t, n, jc): per step one S quad / two exps / AV + d quads.
            # Phase-1 work not needed up front is drip-fed one chunk per step
            # ("extras") so the exp chain starts as early as possible.
            steps = [
                (t, n, jc)
                for t in range(2) for n in range(2) for jc in range(8)
            ]
            av_tiles_all = {}
            d_tiles_all = {}

            def emit_s_half(step, p):
                # 2 score matmuls at distinct row groups -> run concurrently.
                t, n, jc = step
                assert (id(tk), t, 0) in kq_ready and (id(tq), t, n) in kq_ready
                assert jc < 4 or (id(tk), t, 1) in kq_ready
                s_ps = ps_s.tile(
                    [128, 1024], F32, tag="S", name=f"s_{t}_{n}_{jc}_{p}"
                )
                for gg in range(2):
                    g = 2 * p + gg
                    nc.tensor.matmul(
                        s_ps[:, gg * 512:(gg + 1) * 512],
                        tk[32 * g:32 * g + 16, t, jc * 128:(jc + 1) * 128],
                        tq[32 * g:32 * g + 16, t, n * 512:(n + 1) * 512],
                        start=True, stop=True,
                        tile_position=(32 * g, 0),
                    )
                return s_ps

            def finish_tn(t, n):
                # R = 1/d (d ~ L, far from reciprocal_approx edge cases)
                av_ps = av_tiles_all.pop((t, n))
                d_ps = d_tiles_all.pop((t, n))
                rb_sb = wpool.tile([128, 512], F32, tag="rb")
                nc.vector.reciprocal_approx_fast(rb_sb[:], d_ps[:])
                nc.vector.tensor_tensor(
                    ybf[:, t, n * 512:(n + 1) * 512], av_ps[:], rb_sb[:],
                    ALU.mult,
                )
                if peacc_done[t]:
                    emit_pe_add(t, n)
                else:
                    pe_pending.append((t, n))

            # minimal front for step (0, 0, 0): the first S quad only reads
            # tk columns 0:128 (j-chunk 0), so produce just those first (a
            # cheap N=128 matmul), then full tq; the rest of tk's n=0 half
            # follows behind the S quad.
            # tq first (the longer matmul chain) with its two kc matmuls
            # split so kc=0 streams while xa's kc=1 half is still landing;
            # bias on the idle ScalarE (table already loaded); tk0a bias on
            # VectorE so both biases run concurrently.
            kq_ready.add((id(tq), 0, 0))
            ps = misc_ps()
            for kc in range(2):
                nc.tensor.matmul(
                    ps[:], wkq(0)[:, kc, 128:256], xa[:, kc, :],
                    start=(kc == 0), stop=(kc == 1),
                )
            nc.scalar.activation(
                tq[:, 0, :512], ps[:], AF.Identity, bias=bq[:, 0:1],
            )
            kq_ready.add((id(tk), 0, 0))
            ps = misc_ps()
            for kc in range(2):
                nc.tensor.matmul(
                    ps[:, :128], wkq(0)[:, kc, :128], xa[:, kc, :128],
                    start=(kc == 0), stop=(kc == 1),
                )
            nc.vector.tensor_scalar(
                tk[:, 0, :128], ps[:, :128], bk[:, 0:1], None, ALU.add,
            )
            # drip-fed producers; ORDER MATTERS: vT[jc] must be emitted no
            # later than step jc (its AV consumer), tk/tq halves before the
            # S quads that read them (S for step i+1 is emitted during i).
            # Extras marked "sc" run their PSUM->SBUF bias copy on ScalarE --
            # only placed on DVE_STEPS where ScalarE has a free slot.
            extras = {
                0: [lambda: emit_vt(0)],
                1: [lambda: emit_vt(1), lambda: emit_kq(0, 1, "k")],
                2: [lambda: emit_vt(2)],
                3: [lambda: emit_vt(3)],
                4: [lambda: emit_vt(4)],
                5: [lambda: emit_vt(5), lambda: emit_kq(0, 1, "q", True)],
                6: [lambda: emit_vt(6)],
                7: [lambda: emit_vt(7)],
                9: [lambda: emit_kq(1, 0, "k")],
                10: [lambda: emit_kq(1, 0, "q", True)],
                12: [lambda: emit_kq(1, 1, "k", True)],
                14: [lambda: emit_kq(1, 1, "q", True)],
                16: [lambda: emit_vnat(0, 0)],
                18: [lambda: emit_vnat(0, 1)],
                20: [lambda: emit_peacc(0)],
                21: [lambda: emit_vnat(1, 0)],
                24: [lambda: emit_vnat(1, 1)],
                25: [lambda: emit_peacc(1)],
                # n=0 proj units: both n=0 halves of y are ready after step
                # 23; borrow an S slot, bias on ScalarE, so the whole n=0
                # output (matmul + bias + store) completes mid-stream.
                26: [lambda: emit_proj(0, 0, misc_ps(), nc.scalar)],
                27: [lambda: emit_proj(1, 0, misc_ps(), nc.scalar)],
            }

            s_next = [emit_s_half(steps[0], 0), emit_s_half(steps[0], 1)]
            # tk n=0 columns 128:512 (j-chunks 1-3; their S quads are emitted
            # from step 0 onward, always after this point in the PE stream)
            ps = misc_ps()
            for kc in range(2):
                nc.tensor.matmul(
                    ps[:, :384], wkq(0)[:, kc, :128], xa[:, kc, 128:512],
                    start=(kc == 0), stop=(kc == 1),
                )
            nc.vector.tensor_scalar(
                tk[:, 0, 128:512], ps[:, :384], bk[:, 0:1], None, ALU.add,
            )
            for i, step in enumerate(steps):
                t, n, jc = step
                s_cur = s_next
                use_dve = i in DVE_STEPS
                e_sb = []
                for p in range(2):
                    e = epool.tile([128, 1024], BF16, tag="E", name=f"e{i}_{p}")
                    if p == 1 and use_dve:
                        nc.vector.tensor_scalar(
                            e[:].bitcast(I16), s_cur[p][:],
                            SCH_A, SCH_B, ALU.mult, ALU.add,
                        )
                    else:
                        nc.scalar.activation(e[:], s_cur[p][:], AF.Exp, scale=SCALE)
                    e_sb.append(e)
                if (t, n) not in av_tiles_all:
                    av_tiles_all[(t, n)] = ps_av.tile(
                        [128, 512], F32, tag="av", name=f"av_{t}_{n}"
                    )
                    d_tiles_all[(t, n)] = ps_d.tile(
                        [128, 512], F32, tag="d", name=f"d_{t}_{n}"
                    )
                av_ps = av_tiles_all[(t, n)]
                d_ps = d_tiles_all[(t, n)]
                # Front HAM duty filler (see the tail filler below): covers
                # the PE's dead window at the start of each step while this
                # step's exps are still in flight.  Only while the d group
                # is open mid-accumulation (jc 1..6) so it can't race the
                # group's start/stop or the finish reader.
                if 1 <= jc <= 6:
                    nc.tensor.matmul(
                        d_ps[0:32, :], scratch[:, :32], xa[:, 0, :],
                        start=False, stop=False,
                        tile_position=(0, 0),
                        skip_group_check=True,
                    )
                # extras next: they have no exp dependencies either, so the
                # PE runs them while this step's exps are still in flight.
                for fn in extras.pop(i, []):
                    fn()
                assert jc in vt_ready, (t, n, jc)

                def quads(gs):
                    for g in gs:
                        # denominator, pre-broadcast: ones(128,32)^T @ E
                        # fills all 32 partitions of head g with d_h[i].
                        # Before the AV matmuls so the tail's reciprocal
                        # starts earlier.
                        nc.tensor.matmul(
                            d_ps[32 * g:32 * g + 32, :],
                            onesb[:, :32],
                            e_sb[g // 2][:, (g % 2) * 512:(g % 2 + 1) * 512],
                            start=(jc == 0), stop=(jc == 7),
                            tile_position=(0, 32 * g),
                            skip_group_check=True,
                        )
                    for g in gs:
                        h = 4 * t + g
                        nc.tensor.matmul(
                            av_ps[32 * g:32 * g + 32, :],
                            vT[:, jc, 32 * h:32 * h + 32],
                            e_sb[g // 2][:, (g % 2) * 512:(g % 2 + 1) * 512],
                            start=(jc == 0), stop=(jc == 7),
                            tile_position=(0, 32 * g),
                            skip_group_check=True,
                        )

                # PE emission order per step type, chosen so the PE stream
                # packs FIFO-tight behind each exp's completion:
                #  - DVE steps (p1 exp ends early on VectorE): S halves for
                #    the next step, then full-width quads (4-way column
                #    concurrency).
                #  - ScalarE-only steps (p1 exp ends a full EXP later): the
                #    p0-dependent quad half fills the gap before S-p1.
                nxt = steps[i + 1] if i + 1 < len(steps) else None
                if nxt is None:
                    quads([0, 1])
                    quads([2, 3])
                elif use_dve:
                    s_next = [emit_s_half(nxt, 0), emit_s_half(nxt, 1)]
                    quads([0, 1, 2, 3])
                else:
                    sA = emit_s_half(nxt, 0)
                    quads([0, 1])
                    s_next = [sA, emit_s_half(nxt, 1)]
                    quads([2, 3])
                # HAM duty filler: one throwaway matmul accumulating +0.0
                # (zero weights, start=False) into the open d accumulator.
                # It has NO cross-engine dependencies -- pure PE-FIFO work at
                # the tail of the step's burst -- so it raises the array's
                # duty cycle (keeping the HAM clock gate at 2.4 GHz, which
                # otherwise oscillates and halves every matmul) without
                # serializing the exp chains.
                if jc < 7:
                    nc.tensor.matmul(
                        d_ps[0:32, :384], scratch[:, :32], xa[:, 0, :384],
                        start=False, stop=False,
                        tile_position=(0, 0),
                        skip_group_check=True,
                    )
                if jc == 7:
                    finish_tn(t, n)
            assert not extras

            # ---- phase 3: remaining (n=1) proj units ----
            # kc=0 (reads the long-finished t=0 half of y) is emitted first so
            # it executes off the critical tail chain; kc=1 + bias + DMA wait
            # only on the last pe add.
            prj_ps = []
            for mo in range(2):
                ps = misc_ps()
                prj_ps.append(ps)
                nc.tensor.matmul(
                    ps[:], wpt[:, 0, mo * 128:(mo + 1) * 128],
                    ybf[:, 0, 512:], start=True, stop=False,
                )
            for mo in range(2):
                ps = prj_ps[mo]
                nc.tensor.matmul(
                    ps[:], wpt[:, 1, mo * 128:(mo + 1) * 128],
                    ybf[:, 1, 512:], start=False, stop=True,
                )
                # biases on different engines, stores from both HWDGE
                # queues, so the two final units finish in parallel
                if mo == 0:
                    nc.scalar.activation(
                        zout[:, mo, 512:], ps[:],
                        AF.Identity, bias=bproj[:, mo:mo + 1],
                    )
                else:
                    nc.vector.tensor_scalar(
                        zout[:, mo, 512:], ps[:],
                        bproj[:, mo:mo + 1], None, ALU.add,
                    )
                dma_q = nc.sync if mo == 0 else nc.scalar
                dma_q.dma_start(
                    out_d.ap()[:, mo, 512:], zout[:, mo, 512:],
                )

    nc.compile()
    return nc


def pack_inputs(x, w_qkv, b_qkv, w_pe, b_pe, w_proj, b_proj):
    """Host-side packing of the full inputs into per-core in_maps."""
    bf16 = ml_dtypes.bfloat16
    f32 = np.float32

    # k/q packed layouts: tile t in {0,1}; partition m = 32*g + r; head h = 4t+g.
    # Only r < 16 is live (k channel r -> qkv row 64h+16+r; q channel r -> 64h+r);
    # r >= 16 columns are zero so both tiles stay 32-aligned per head.
    w_kA = np.zeros((256, 256), dtype=w_qkv.dtype)
    w_qA = np.zeros((256, 256), dtype=w_qkv.dtype)
    b_kP = np.zeros((128, 2), dtype=b_qkv.dtype)
    b_qP = np.zeros((128, 2), dtype=b_qkv.dtype)
    for t in range(2):
        for m in range(128):
            g, r = m // 32, m % 32
            h = 4 * t + g
            if r < 16:
                w_kA[:, t * 128 + m] = w_qkv[64 * h + 16 + r]
                w_qA[:, t * 128 + m] = w_qkv[64 * h + r]
                b_kP[m, t] = b_qkv[64 * h + 16 + r]
                b_qP[m, t] = b_qkv[64 * h + r]

    v_rows = np.array([64 * (c // 32) + 32 + c % 32 for c in range(256)])
    w_v = w_qkv[v_rows].T  # (256 d, 256 c)
    b_v = b_qkv[v_rows]

    def kpart(a):  # (256, F) -> (128, 2, F)
        return np.ascontiguousarray(a.reshape(2, 128, -1).transpose(1, 0, 2))

    def chan2(a):  # (256,) -> (128, 2)
        return np.ascontiguousarray(a.reshape(2, 128).T)

    pf = np.concatenate([
        b_kP, b_qP, chan2(b_v),
        kpart(w_pe[:, 0, :]).reshape(128, 6),
        chan2(b_pe), chan2(b_proj),
    ], axis=1).astype(f32)  # (128, 16)
    bvb = np.broadcast_to(b_v[None, :], (128, 256)).astype(f32)
    pfb = np.ascontiguousarray(np.concatenate([pf, bvb], axis=1))  # (128, 272)

    kA = kpart(w_kA)   # (128, 2, 256): [:, :, t*128:(t+1)*128] is tile t
    qA = kpart(w_qA)
    vA = kpart(w_v)
    pA = kpart(w_proj.T)
    wt0 = np.concatenate([kA[:, :, 0:128], qA[:, :, 0:128]], axis=2)
    wrest = np.concatenate(
        [kA[:, :, 128:256], qA[:, :, 128:256], vA, pA], axis=2
    )
    common = {
        "wt0": np.ascontiguousarray(wt0).astype(bf16),
        "wrest": np.ascontiguousarray(wrest).astype(bf16),
        "pfb": pfb,
    }
    in_maps = []
    for b in range(B):
        m = dict(common)
        xp = kpart(x[b]).astype(bf16)
        m["xa"] = np.ascontiguousarray(xp[:, :, :512])
        m["xb"] = np.ascontiguousarray(xp[:, :, 512:])
        in_maps.append(m)
    return in_maps


_CACHE = {}


def kernel(x, w_qkv, b_qkv, w_pe, b_pe, w_proj, b_proj):
    x = np.asarray(x, dtype=np.float32)
    w_qkv = np.asarray(w_qkv, dtype=np.float32)
    b_qkv = np.asarray(b_qkv, dtype=np.float32)
    w_pe = np.asarray(w_pe, dtype=np.float32)
    b_pe = np.asarray(b_pe, dtype=np.float32)
    w_proj = np.asarray(w_proj, dtype=np.float32)
    b_proj = np.asarray(b_proj, dtype=np.float32)

    if "nc" not in _CACHE:
        _CACHE["nc"] = build_kernel()
    nc = _CACHE["nc"]

    in_maps = pack_inputs(x, w_qkv, b_qkv, w_pe, b_pe, w_proj, b_proj)

    trace = os.environ.get("BASS_KERNEL_TRACE", "") == "1"
    if trace:
        _install_ntff_shim()
    res = run_bass_kernel_spmd(
        nc, in_maps, core_ids=list(range(B)), trace=trace,
    )
    if trace:
        _CACHE["last_result"] = res

    out = np.empty((B, DIM, L), dtype=np.float32)
    for b in range(B):
        z = np.asarray(res.results[b]["out"], dtype=np.float32)  # (128, 2, 1024)
        out[b] = z.transpose(1, 0, 2).reshape(DIM, L)
    return out
